# revision 13
# baseline (speedup 1.0000x reference)
"""Associative-embedding loss kernel for 8 Trainium2 NeuronCores.

Math: per image b, with tl[n,c] = pred[b,c,ty,tx] and br[n,c] = target[b,c,by,bx]
gathered at the N=128 match points:
  pull_b = sum_{n,c} (tl-br)^2 / (2N)
  s[n]   = sum_c (tl+br),  A'[i,j] = s[i]-s[j]   (A = A'/2)
  push_b = (0.5*(sum|A'+2| - sum|A'|) - N) / (N(N-1))
using sum_{ij} relu(1-|A|) = sum|A'+2| - sum|A'| for antisymmetric A'
(the diagonal contributes 2N, removed on the host).

Strategy: data-parallel over B (8 images per core); the host does the
point gather (HW indirect DMA is ~1.3us per index row, so extraction is
host-side) and uploads ONLY the gathered points — every loss FLOP runs
on device.  The kernel is ~17 instructions, one 61KB fp8 upload (40
per-partition descriptors) and one 24B result DMA.

The core trick: A'[i, 128b+j] = s_b[i] - s_b[j] for 4 images is ONE
K=40 matmul per 512-column PSUM bank — no on-device assembly at all.
Each 768-column slice of the upload carries what one bank contracts:
  lhsT W [41,128]: rows 0:16 tl[b,:,c] (row 4b+c), rows 16:32 br,
      rows 32:40 = -1, row 40 = 2            (all uploaded)
  rhs [40,512]: rows 0:32 block-diagonal indicator (row 4b+c is 1 on
      column block b), rows 32:40 the same values in flat layout
      (row q = corner/channel q, column 128b+j = point j of image b)
  out[i, 128b+j] = sum_c tl[b,i,c] + sum_c br[b,i,c]     (indicator)
                   - sum_q v[q, 128b+j]                   (-1 rows)
                 = s_b[i] - s_b[j]
i.e. the per-point channel sums s are computed inline by the
contraction itself.  Each reducer engine owns one bank outright (no
cross-engine PSUM bank handoffs): ScalarE row-reduces |A'+2| and |A'|
on bank A (Abs with bias 2 / bias 0 via accum_out, main out to a junk
PSUM bank — ScE writes PSUM faster than SBUF), VectorE row-reduces
|A'| (abs reduce) and |A'+2| ((x+2) abs_max 0 tensor_scalar) on bank
B; pull is a DVE subtract + square-accumulate per half.  The six
per-core partials fold to one [1,6] f32 row via two ones-vector
matmuls (partition reduction + transpose in one PE op) so the result
DMA is a single descriptor.  fp8e4 uploads only perturb the result
~1.6e-3, far inside the 2e-2 gate (the indicator and -1 constants are
exact in fp8).
"""

import numpy as np

B, C, H, W, N = 64, 4, 256, 256, 128
M = 8            # cores
BL = B // M      # images per core
HC = 512 + 2 * N  # columns per upload half

_GRAPH = None

# block-diagonal indicator for one bank: row 4b+c is 1 on column block b
_INDH = np.repeat(np.kron(np.eye(4, dtype=np.float32),
                          np.ones((1, N), np.float32)), 4, axis=0)  # [16, 512]


def _build_graph():
    import concourse.bacc as bacc
    import concourse.mybir as mybir
    from concourse.tile import TileContext

    f32 = mybir.dt.float32
    bf16 = mybir.dt.bfloat16
    fp8 = mybir.dt.float8e4
    Alu = mybir.AluOpType
    Act = mybir.ActivationFunctionType
    Axis = mybir.AxisListType

    nc = bacc.Bacc()
    # two 768-col halves side by side; per half:
    # rhs cols 0:512 | lhsT W cols 512:640 | br cols 640:768 (pull path)
    m_d = nc.declare_dram_parameter("m", [41, 2 * HC], fp8, isOutput=False)
    o_d = nc.declare_dram_parameter("o", [1, 8], f32, isOutput=True)

    with TileContext(nc) as tc:
        with (
            tc.tile_pool(name="sb", bufs=1) as pool,
            tc.tile_pool(name="ps", bufs=1, space="PSUM") as psum,
        ):
            mg = pool.tile([41, 2 * HC], fp8)
            nc.sync.dma_start(out=mg[:], in_=m_d[:])

            # constants while the DMA flies (all DVE: no extra engines)
            ones = pool.tile([128, 1], f32)
            nc.vector.memset(ones[:], 1.0)
            two = pool.tile([128, 1], f32)
            nc.vector.memset(two[:], 2.0)
            acc = pool.tile([128, 4], f32)   # pullA, pullB, |A'|B, |A'+2|B
            nc.vector.memset(acc[:], 0.0)
            accs = pool.tile([128, 2], f32)  # |A'+2|A, |A'|A (scalar engine)

            # pull: d = tl - br per half, square-accumulate (DVE)
            dA = pool.tile([16, N], f32)
            nc.vector.tensor_sub(dA[:], mg[0:16, 512:512 + N],
                                 mg[0:16, 512 + N:HC])
            d2A = pool.tile([16, N], f32)
            nc.vector.scalar_tensor_tensor(
                out=d2A[:], in0=dA[:], scalar=0.0, in1=dA[:],
                op0=Alu.bypass, op1=Alu.mult, accum_out=acc[0:16, 0:1])
            dB = pool.tile([16, N], f32)
            nc.vector.tensor_sub(dB[:], mg[0:16, HC + 512:HC + 512 + N],
                                 mg[0:16, HC + 512 + N:2 * HC])
            d2B = pool.tile([16, N], f32)
            nc.vector.scalar_tensor_tensor(
                out=d2B[:], in0=dB[:], scalar=0.0, in1=dB[:],
                op0=Alu.bypass, op1=Alu.mult, accum_out=acc[0:16, 1:2])

            # A'[i, 128b+j] = s_b[i] - s_b[j]: one K=40 matmul per bank
            bankA = psum.tile([128, 512], f32, name="bankA", tag="a")
            bankB = psum.tile([128, 512], f32, name="bankB", tag="b")
            bankB2 = psum.tile([128, 512], f32, name="bankB2", tag="b2")
            nc.tensor.matmul(out=bankA[:], lhsT=mg[0:40, 512:512 + N],
                             rhs=mg[0:40, 0:512], start=True, stop=True)
            nc.tensor.matmul(out=bankB[:], lhsT=mg[0:40, HC + 512:HC + 512 + N],
                             rhs=mg[0:40, HC:HC + 512], start=True, stop=True)
            # same columns with the K=41 extra row included: A' + 2
            nc.tensor.matmul(out=bankB2[:], lhsT=mg[0:41, HC + 512:HC + 512 + N],
                             rhs=mg[0:41, HC:HC + 512], start=True, stop=True)

            # row reductions, one engine per bank (no cross-engine PSUM
            # bank handoffs): ScalarE takes bank A (ready first, ScE's
            # two passes are longer), VectorE takes banks B/B2.
            scr = psum.tile([128, 512], f32, name="scr", tag="scr")
            nc.scalar.activation(
                out=scr[:], in_=bankA[:], func=Act.Abs,
                bias=two[:, 0:1], scale=1.0, accum_out=accs[:, 0:1])
            nc.scalar.activation(
                out=scr[:], in_=bankA[:], func=Act.Abs,
                bias=0.0, scale=1.0, accum_out=accs[:, 1:2])
            nc.vector.tensor_reduce(
                out=acc[:, 2:3], in_=bankB[:], axis=Axis.X,
                op=Alu.add, apply_absolute_value=True)
            nc.vector.tensor_reduce(
                out=acc[:, 3:4], in_=bankB2[:], axis=Axis.X,
                op=Alu.add, apply_absolute_value=True)

            # fold the 128-partition partials to one row: out[0, k] =
            # sum_p acc[p, k] — partition reduction + transpose in one
            # matmul, so the result DMA is a single tiny descriptor.
            # accs (ScalarE) finishes first, so its fold goes first.
            pr = psum.tile([1, 8], f32, name="pr", tag="pr")
            nc.tensor.matmul(out=pr[0:1, 4:6], lhsT=ones[:], rhs=accs[:],
                             start=True, stop=True)
            nc.tensor.matmul(out=pr[0:1, 0:4], lhsT=ones[:], rhs=acc[:],
                             start=True, stop=True)
            res = pool.tile([1, 8], f32)
            nc.vector.tensor_copy(res[:], pr[:])
            nc.sync.dma_start(out=o_d[:], in_=res[:])
    nc.finalize()
    return nc


def _get_graph():
    global _GRAPH
    if _GRAPH is None:
        _GRAPH = _build_graph()
    return _GRAPH


def _half(tls, brs):
    """Build one [41, HC] upload half from tl/br gathers of 4 images."""
    K4 = 4 * N
    m = np.zeros((41, HC), np.float32)
    m[40, 0:512] = 1.0          # rhs row 40: +2 bias partner
    m[40, 512:512 + N] = 2.0    # lhsT row 40
    m[0:16, 0:512] = _INDH
    m[16:32, 0:512] = _INDH
    m[32:36, 0:512] = tls.transpose(2, 0, 1).reshape(4, K4)
    m[36:40, 0:512] = brs.transpose(2, 0, 1).reshape(4, K4)
    m[0:16, 512:512 + N] = tls.transpose(0, 2, 1).reshape(16, N)
    m[16:32, 512:512 + N] = brs.transpose(0, 2, 1).reshape(16, N)
    m[32:40, 512:512 + N] = -1.0
    m[0:16, 512 + N:HC] = brs.transpose(0, 2, 1).reshape(16, N)
    return m


def _make_in_maps(pred, target, match):
    import ml_dtypes

    fp8 = ml_dtypes.float8_e4m3
    barr = np.arange(B)[:, None]
    tl = pred[barr, :, match[:, :, 0, 0], match[:, :, 0, 1]]    # [B, N, C]
    br = target[barr, :, match[:, :, 1, 0], match[:, :, 1, 1]]  # [B, N, C]

    in_maps = []
    for i in range(M):
        s0 = i * BL
        m = np.concatenate(
            [_half(tl[s0:s0 + 4], br[s0:s0 + 4]),
             _half(tl[s0 + 4:s0 + 8], br[s0 + 4:s0 + 8])], axis=1)
        in_maps.append({"m": m.astype(fp8)})
    return in_maps


def _finish(core_outs):
    pull_total = 0.0
    m_total = 0.0
    for o in core_outs:
        o = np.asarray(o, dtype=np.float64)
        pull_total += o[0, 0] + o[0, 1]
        m_total += o[0, 3] + o[0, 4] - o[0, 2] - o[0, 5]
    # per image: 0.5*(sum|A'+2| - sum|A'|) = P_b + N
    pull_all = 0.25 * pull_total / (2 * N)
    push_all = 0.25 * (0.5 * m_total - B * N) / (N * (N - 1))
    return (np.float32(pull_all), np.float32(push_all))


def kernel(pred, target, match):
    from concourse.bass_utils import run_bass_kernel_spmd

    nc = _get_graph()
    in_maps = _make_in_maps(np.asarray(pred), np.asarray(target), np.asarray(match))
    res = run_bass_kernel_spmd(nc, in_maps, core_ids=list(range(M)))
    return _finish([r["o"] for r in res.results])


# revision 14
# speedup vs baseline: 1.0331x; 1.0331x over previous
"""Associative-embedding loss kernel for 8 Trainium2 NeuronCores.

Math: per image b, with tl[n,c] = pred[b,c,ty,tx] and br[n,c] = target[b,c,by,bx]
gathered at the N=128 match points:
  pull_b = sum_{n,c} (tl-br)^2 / (2N)
  s[n]   = sum_c (tl+br),  A'[i,j] = s[i]-s[j]   (A = A'/2)
  push_b = (0.5*(sum|A'+2| - sum|A'|) - N) / (N(N-1))
using sum_{ij} relu(1-|A|) = sum|A'+2| - sum|A'| for antisymmetric A'
(the diagonal contributes 2N, removed on the host).

Strategy: data-parallel over B (8 images per core); the host does the
point gather (HW indirect DMA is ~1.3us per index row, so extraction is
host-side) and uploads ONLY the gathered points — every loss FLOP runs
on device.  The kernel is ~18 instructions, two parallel ~31KB fp8
uploads and one 24B result DMA (a single merged upload measured 2.8us
slower: one HWDGE queue drains descriptors far slower than two).

The core trick: A'[i, 128b+j] = s_b[i] - s_b[j] for 4 images is ONE
K=40 matmul per 512-column PSUM bank — no on-device assembly at all.
Each 768-column slice of the upload carries what one bank contracts:
  lhsT W [41,128]: rows 0:16 tl[b,:,c] (row 4b+c), rows 16:32 br,
      rows 32:40 = -1, row 40 = 2            (all uploaded)
  rhs [40,512]: rows 0:32 block-diagonal indicator (row 4b+c is 1 on
      column block b), rows 32:40 the same values in flat layout
      (row q = corner/channel q, column 128b+j = point j of image b)
  out[i, 128b+j] = sum_c tl[b,i,c] + sum_c br[b,i,c]     (indicator)
                   - sum_q v[q, 128b+j]                   (-1 rows)
                 = s_b[i] - s_b[j]
i.e. the per-point channel sums s are computed inline by the
contraction itself.  Each reducer engine owns one bank outright (no
cross-engine PSUM bank handoffs): ScalarE row-reduces |A'+2| and |A'|
on bank A (Abs with bias 2 / bias 0 via accum_out, main out to a junk
PSUM bank — ScE writes PSUM faster than SBUF), VectorE row-reduces
|A'| (abs reduce) and |A'+2| ((x+2) abs_max 0 tensor_scalar) on bank
B; pull is a DVE subtract + square-accumulate per half.  The six
per-core partials fold to one [1,6] f32 row via two ones-vector
matmuls (partition reduction + transpose in one PE op) so the result
DMA is a single descriptor.  fp8e4 uploads only perturb the result
~1.6e-3, far inside the 2e-2 gate (the indicator and -1 constants are
exact in fp8).
"""

import numpy as np

B, C, H, W, N = 64, 4, 256, 256, 128
M = 8            # cores
BL = B // M      # images per core
HC = 512 + 2 * N  # columns per upload half

_GRAPH = None

# block-diagonal indicator for one bank: row 4b+c is 1 on column block b
_INDH = np.repeat(np.kron(np.eye(4, dtype=np.float32),
                          np.ones((1, N), np.float32)), 4, axis=0)  # [16, 512]


def _build_graph():
    import concourse.bacc as bacc
    import concourse.mybir as mybir
    from concourse.tile import TileContext

    f32 = mybir.dt.float32
    bf16 = mybir.dt.bfloat16
    fp8 = mybir.dt.float8e4
    Alu = mybir.AluOpType
    Act = mybir.ActivationFunctionType
    Axis = mybir.AxisListType

    nc = bacc.Bacc()
    # per half: rhs cols 0:512 | lhsT W cols 512:640 | br cols 640:768
    # (pull path); m2 carries the extra K=41 row for the A'+2 bank
    m1_d = nc.declare_dram_parameter("m1", [40, HC], fp8, isOutput=False)
    m2_d = nc.declare_dram_parameter("m2", [41, HC], fp8, isOutput=False)
    o_d = nc.declare_dram_parameter("o", [1, 8], f32, isOutput=True)

    with TileContext(nc) as tc:
        with (
            tc.tile_pool(name="sb", bufs=1) as pool,
            tc.tile_pool(name="ps", bufs=1, space="PSUM") as psum,
        ):
            mg = pool.tile([40, HC], fp8)
            nc.sync.dma_start(out=mg[:], in_=m1_d[:])
            mh = pool.tile([41, HC], fp8)
            nc.scalar.dma_start(out=mh[:], in_=m2_d[:])

            # constants while the DMA flies (all DVE: no extra engines)
            ones = pool.tile([128, 1], f32)
            nc.vector.memset(ones[:], 1.0)
            two = pool.tile([128, 1], f32)
            nc.vector.memset(two[:], 2.0)
            acc = pool.tile([128, 4], f32)   # pullA, pullB, |A'|B, |A'+2|B
            nc.vector.memset(acc[:], 0.0)
            accs = pool.tile([128, 2], f32)  # |A'+2|A, |A'|A (scalar engine)

            # pull: d = tl - br per half, square-accumulate (DVE)
            dA = pool.tile([16, N], f32)
            nc.vector.tensor_sub(dA[:], mg[0:16, 512:512 + N],
                                 mg[0:16, 512 + N:HC])
            d2A = pool.tile([16, N], f32)
            nc.vector.scalar_tensor_tensor(
                out=d2A[:], in0=dA[:], scalar=0.0, in1=dA[:],
                op0=Alu.bypass, op1=Alu.mult, accum_out=acc[0:16, 0:1])
            dB = pool.tile([16, N], f32)
            nc.vector.tensor_sub(dB[:], mh[0:16, 512:512 + N],
                                 mh[0:16, 512 + N:HC])
            d2B = pool.tile([16, N], f32)
            nc.vector.scalar_tensor_tensor(
                out=d2B[:], in0=dB[:], scalar=0.0, in1=dB[:],
                op0=Alu.bypass, op1=Alu.mult, accum_out=acc[0:16, 1:2])

            # A'[i, 128b+j] = s_b[i] - s_b[j]: one K=40 matmul per bank
            bankA = psum.tile([128, 512], f32, name="bankA", tag="a")
            bankB = psum.tile([128, 512], f32, name="bankB", tag="b")
            bankB2 = psum.tile([128, 512], f32, name="bankB2", tag="b2")
            nc.tensor.matmul(out=bankA[:], lhsT=mg[:, 512:512 + N],
                             rhs=mg[:, 0:512], start=True, stop=True)
            nc.tensor.matmul(out=bankB[:], lhsT=mh[0:40, 512:512 + N],
                             rhs=mh[0:40, 0:512], start=True, stop=True)
            # same columns with the K=41 extra row included: A' + 2
            nc.tensor.matmul(out=bankB2[:], lhsT=mh[0:41, 512:512 + N],
                             rhs=mh[0:41, 0:512], start=True, stop=True)

            # row reductions, one engine per bank (no cross-engine PSUM
            # bank handoffs): ScalarE takes bank A (ready first, ScE's
            # two passes are longer), VectorE takes banks B/B2.
            scr = psum.tile([128, 512], f32, name="scr", tag="scr")
            nc.scalar.activation(
                out=scr[:], in_=bankA[:], func=Act.Abs,
                bias=two[:, 0:1], scale=1.0, accum_out=accs[:, 0:1])
            nc.scalar.activation(
                out=scr[:], in_=bankA[:], func=Act.Abs,
                bias=0.0, scale=1.0, accum_out=accs[:, 1:2])
            nc.vector.tensor_reduce(
                out=acc[:, 2:3], in_=bankB[:], axis=Axis.X,
                op=Alu.add, apply_absolute_value=True)
            nc.vector.tensor_reduce(
                out=acc[:, 3:4], in_=bankB2[:], axis=Axis.X,
                op=Alu.add, apply_absolute_value=True)

            # fold the 128-partition partials to one row: out[0, k] =
            # sum_p acc[p, k] — partition reduction + transpose in one
            # matmul, so the result DMA is a single tiny descriptor.
            # accs (ScalarE) finishes first, so its fold goes first.
            pr = psum.tile([1, 8], f32, name="pr", tag="pr")
            nc.tensor.matmul(out=pr[0:1, 4:6], lhsT=ones[:], rhs=accs[:],
                             start=True, stop=True)
            nc.tensor.matmul(out=pr[0:1, 0:4], lhsT=ones[:], rhs=acc[:],
                             start=True, stop=True)
            res = pool.tile([1, 8], f32)
            nc.vector.tensor_copy(res[:], pr[:])
            nc.sync.dma_start(out=o_d[:], in_=res[:])
    nc.finalize()
    return nc


def _get_graph():
    global _GRAPH
    if _GRAPH is None:
        _GRAPH = _build_graph()
    return _GRAPH


def _half(tls, brs):
    """Build one [41, HC] upload half from tl/br gathers of 4 images."""
    K4 = 4 * N
    m = np.zeros((41, HC), np.float32)
    m[40, 0:512] = 1.0          # rhs row 40: +2 bias partner (half 2 only)
    m[40, 512:512 + N] = 2.0    # lhsT row 40
    m[0:16, 0:512] = _INDH
    m[16:32, 0:512] = _INDH
    m[32:36, 0:512] = tls.transpose(2, 0, 1).reshape(4, K4)
    m[36:40, 0:512] = brs.transpose(2, 0, 1).reshape(4, K4)
    m[0:16, 512:512 + N] = tls.transpose(0, 2, 1).reshape(16, N)
    m[16:32, 512:512 + N] = brs.transpose(0, 2, 1).reshape(16, N)
    m[32:40, 512:512 + N] = -1.0
    m[0:16, 512 + N:HC] = brs.transpose(0, 2, 1).reshape(16, N)
    return m


def _make_in_maps(pred, target, match):
    import ml_dtypes

    fp8 = ml_dtypes.float8_e4m3
    barr = np.arange(B)[:, None]
    tl = pred[barr, :, match[:, :, 0, 0], match[:, :, 0, 1]]    # [B, N, C]
    br = target[barr, :, match[:, :, 1, 0], match[:, :, 1, 1]]  # [B, N, C]

    in_maps = []
    for i in range(M):
        s0 = i * BL
        m1 = _half(tl[s0:s0 + 4], br[s0:s0 + 4])[0:40]
        m2 = _half(tl[s0 + 4:s0 + 8], br[s0 + 4:s0 + 8])
        in_maps.append({"m1": m1.astype(fp8), "m2": m2.astype(fp8)})
    return in_maps


def _finish(core_outs):
    pull_total = 0.0
    m_total = 0.0
    for o in core_outs:
        o = np.asarray(o, dtype=np.float64)
        pull_total += o[0, 0] + o[0, 1]
        m_total += o[0, 3] + o[0, 4] - o[0, 2] - o[0, 5]
    # per image: 0.5*(sum|A'+2| - sum|A'|) = P_b + N
    pull_all = 0.25 * pull_total / (2 * N)
    push_all = 0.25 * (0.5 * m_total - B * N) / (N * (N - 1))
    return (np.float32(pull_all), np.float32(push_all))


def kernel(pred, target, match):
    from concourse.bass_utils import run_bass_kernel_spmd

    nc = _get_graph()
    in_maps = _make_in_maps(np.asarray(pred), np.asarray(target), np.asarray(match))
    res = run_bass_kernel_spmd(nc, in_maps, core_ids=list(range(M)))
    return _finish([r["o"] for r in res.results])
